# revision 24
# baseline (speedup 1.0000x reference)
"""AttnBlock (GroupNorm -> single-head attention over 64x64 tokens -> proj -> residual)
for Trainium2, SPMD over 8 NeuronCores.

Sharding: core = batch(4) x query-half(2).  Each core receives x[b] with its
query half rotated to the front (token order along j is permutation-invariant
for softmax-attention and for GroupNorm stats), computes GroupNorm + k/vT over
all 4096 tokens, q over its 2048 tokens, streaming-softmax attention without
max-subtraction, and the output projection + residual for its 2048 tokens.

All matmuls run in fp8(e4m3) with DoubleRow perf mode: each instruction
contracts 2x128=256 inputs at 0.5 cycles/row -> 4x bf16 matmul throughput.
Scales (all exact powers of two, folded away):
  weights stored as 16*W^T fp8; q,k stored as 16*q (bias 16*b folded in)
  S_psum = 256*(q.k);  et = exp(S/sqrt(C) - ln16)   (max ~92 < 240 fp8e4 max)
  l_psum = 0.25*sum(et) = sum(e^S)/64; lrb = recip = 64/sum(e^S) broadcast
           to 128 partitions via a tiny ones-matmul (no DRAM roundtrip)
  o8 = O_psum*lrb = 64*(attention out, pre-proj); v has NO bias on device --
       bv is folded host-side into bp' = bp + Wp@bv
  proj_psum = (16Wp)*(o8) = 1024*h_attn;  y = proj*2^-10 + (x + bp')
Residual uses the bf16 x already in SBUF (no f32 x load).

Engine budget per core (cost-model): PE ~82us (was 303), ACT = exp only
~77us, DVE ~60us (GN stats + k-bias + O-normalize), Pool ~46us (h8/v/q
drains + residual combine), SP = DMA ~22us.
"""

import math
import numpy as np
import ml_dtypes

import concourse.bass as bass
import concourse.mybir as mybir
import concourse.tile as tile

P = 128
C = 512
NCC = C // P          # 4 channel chunks
NP2 = NCC // 2        # 2 channel-chunk pairs (DoubleRow)
HW = 4096             # tokens per batch image
IHALF = 2048          # query tokens per core
NBLK = IHALF // 512   # 4 i-blocks of 512
NJC = HW // P         # 32 j chunks of 128
NJP = NJC // 2        # 16 j-chunk pairs
NJT = HW // 512       # 8 j tiles of 512
GS = 16               # channels per group
EPS = 1e-6
WS = 16.0             # host-side weight scale (power of two)
SCALE_S = 1.0 / (WS * WS * math.sqrt(C))
EXP_BIAS = -math.log(16.0)
ONES_VAL = 0.25       # l_psum = sum(e^S)/64 -> recip = 64/sum = o8 scale
PROJ_SCALE = 1.0 / 1024.0

F32 = mybir.dt.float32
BF16 = mybir.dt.bfloat16
FP8 = mybir.dt.float8e4
BF = ml_dtypes.bfloat16
E4 = ml_dtypes.float8_e4m3
DR = mybir.MatmulPerfMode.DoubleRow
ALU = mybir.AluOpType
ACTF = mybir.ActivationFunctionType


def _split_excess_waits(nc):
    """walrus in this container accepts only ONE sync-wait per instruction;
    move extra waits onto same-engine NOPs placed immediately before."""
    for fn in nc.m.functions:
        for bb in fn.blocks:
            insts = list(bb.instructions)
            out = []
            changed = False
            for inst in insts:
                si = inst.sync_info
                if si is not None and len(si.on_wait) > 1:
                    waits = list(si.on_wait)
                    for k, w in enumerate(waits[:-1]):
                        nop = mybir.InstNoOp(
                            name=f"{inst.name}-ws{k}",
                            sync_info=mybir.SyncInfo(on_wait=[w], on_update=[]),
                            bass_nofuse=True,
                            engine=inst.engine,
                        )
                        out.append(nop)
                    inst.sync_info = mybir.SyncInfo(
                        on_wait=[waits[-1]], on_update=list(si.on_update)
                    )
                    changed = True
                out.append(inst)
            if changed:
                bb.instructions = out


def build_nc(split_waits=True):
    nc = bass.Bass()

    x_d = nc.declare_dram_parameter("x_bf", [C, HW], BF16, isOutput=False)
    wq_d = nc.declare_dram_parameter("wq8", [C, C], FP8, isOutput=False)
    wk_d = nc.declare_dram_parameter("wk8", [C, C], FP8, isOutput=False)
    wv_d = nc.declare_dram_parameter("wv8", [C, C], FP8, isOutput=False)
    wp_d = nc.declare_dram_parameter("wp8", [C, C], FP8, isOutput=False)
    # packed per-channel constants: bq16, bk16, bp', gamma, beta (NCC each)
    # then ind16 (P//GS cols)
    consts_d = nc.declare_dram_parameter("consts", [P, 5 * NCC + P // GS], F32,
                                         isOutput=False)
    bcast16_d = nc.declare_dram_parameter("bcast16", [P // GS, P], F32,
                                          isOutput=False)
    ones8_d = nc.declare_dram_parameter("ones8", [P, 2, 1], FP8, isOutput=False)
    y_d = nc.declare_dram_parameter("yout", [C, IHALF], F32, isOutput=True)

    with tile.TileContext(nc) as tc:
        with (
            tc.tile_pool(name="w", bufs=1) as wpool,
            tc.tile_pool(name="const", bufs=1) as cpool,
            tc.tile_pool(name="xb", bufs=1) as xpool,
            tc.tile_pool(name="h8p", bufs=1) as hpool,
            tc.tile_pool(name="k8p", bufs=1) as kpool,
            tc.tile_pool(name="q8p", bufs=1) as qpool,
            tc.tile_pool(name="v8p", bufs=1) as vpool,
        ):
            wq8 = wpool.tile([P, NCC, C], FP8, tag="wq8")
            wk8 = wpool.tile([P, NCC, C], FP8, tag="wk8")
            wv8 = wpool.tile([P, NCC, C], FP8, tag="wv8")
            wp8 = wpool.tile([P, NCC, C], FP8, tag="wp8")

            consts = cpool.tile([P, 5 * NCC + P // GS], F32, tag="consts")
            bq16 = consts[:, 0 * NCC:1 * NCC]
            bk16 = consts[:, 1 * NCC:2 * NCC]
            bppc = consts[:, 2 * NCC:3 * NCC]
            gamma = consts[:, 3 * NCC:4 * NCC]
            beta = consts[:, 4 * NCC:5 * NCC]
            ind16 = consts[:, 5 * NCC:]
            bcast16 = cpool.tile([P // GS, P], F32, tag="bcast16")
            ones8 = cpool.tile([P, 2, 1], FP8, tag="ones8")
            ones_bf = cpool.tile([1, P], BF16, tag="onesbf")
            eps_sb = cpool.tile([P // GS, 1], F32, tag="eps")
            ebias = cpool.tile([P, 1], F32, tag="ebias")

            x_sb = xpool.tile([P, NCC, HW], BF16, tag="x")
            h8 = hpool.tile([P, NCC, HW], FP8, tag="h8")
            k8 = kpool.tile([P, NCC, HW], FP8, tag="k8")
            q8 = qpool.tile([P, NCC, IHALF], FP8, tag="q8")
            vt8 = vpool.tile([P, NJC, C], FP8, tag="vt8")

            # ---- DMAs: x chunks on sync/gpsimd/scalar; weights+consts follow
            half = HW // 2
            for ci, eng in ((0, nc.sync), (1, nc.gpsimd), (2, nc.scalar),
                            (3, nc.sync)):
                eng.dma_start(out=x_sb[:, ci, :half], in_=x_d[ci * P:(ci + 1) * P, :half])
                eng.dma_start(out=x_sb[:, ci, half:], in_=x_d[ci * P:(ci + 1) * P, half:])
            for t, d in ((wq8, wq_d), (wk8, wk_d), (wv8, wv_d), (wp8, wp_d)):
                nc.sync.dma_start(out=t[:], in_=d[:].rearrange("(cc p) o -> p cc o", p=P))
            nc.gpsimd.dma_start(out=consts[:], in_=consts_d[:])
            nc.gpsimd.dma_start(out=bcast16[:], in_=bcast16_d[:])
            nc.gpsimd.dma_start(out=ones8[:], in_=ones8_d[:])
            nc.vector.memset(ones_bf[:], 1.0)
            nc.vector.memset(eps_sb[:], EPS)
            nc.vector.memset(ebias[:], EXP_BIAS)
            sqwarm = cpool.tile([P // GS, 1], F32, tag="sqwarm")
            nc.scalar.activation(
                out=sqwarm[:], in_=eps_sb[:], func=ACTF.Sqrt, scale=1.0,
            )

            # ====== GroupNorm ======
            # stats: DVE bn_stats for chunks 0,1,3; ACT copy/square-accum for
            # chunk 2 (runs in parallel with DVE, ACT is idle pre-attention).
            # h8 = x*sc+sh -> fp8: chunks 2,0,1 on Pool, chunk 3 on DVE, so
            # both DoubleRow chunk-pairs (0,1) and (2,3) complete ~21.5us.
            with (
                tc.tile_pool(name="gn", bufs=2) as gpool,
                tc.tile_pool(name="gnp", bufs=2, space="PSUM") as gpsum_pool,
            ):
                gpsum = gpsum_pool.tile([P // GS, 2 * NCC], F32, tag="gstat")
                sc_all = gpool.tile([P, NCC], F32, tag="scall")
                sh_all = gpool.tile([P, NCC], F32, tag="shall")

                def finish_chunk(ci, t2):
                    nc.tensor.matmul(
                        gpsum[:, ci * 2:(ci + 1) * 2], lhsT=ind16, rhs=t2[:],
                        start=True, stop=True,
                    )
                    # group mean / rstd -> per-channel scale/shift
                    gmr = gpool.tile([P // GS, 2], F32, tag="gmr", name=f"gmr{ci}")
                    nc.vector.tensor_copy(out=gmr[:], in_=gpsum[:, ci * 2:(ci + 1) * 2])
                    mu = gmr[:, 0:1]
                    var = gmr[:, 1:2]
                    tmpv = gpool.tile([P // GS, 1], F32, tag="tmpv")
                    nc.vector.tensor_tensor(tmpv[:], mu, mu, ALU.mult)
                    nc.vector.tensor_tensor(var, var, tmpv[:], ALU.subtract)
                    nc.scalar.activation(
                        out=var, in_=var, func=ACTF.Sqrt, bias=eps_sb[:], scale=1.0,
                    )
                    nc.vector.reciprocal(out=var, in_=var)
                    bpsum = gpsum_pool.tile([P, 2], F32, tag="bc")
                    nc.tensor.matmul(
                        bpsum[:], lhsT=bcast16[:], rhs=gmr[:], start=True, stop=True,
                    )
                    sc = sc_all[:, ci:ci + 1]
                    sh = sh_all[:, ci:ci + 1]
                    nc.vector.tensor_tensor(sc, bpsum[:, 1:2], gamma[:, ci:ci + 1], ALU.mult)
                    nc.vector.tensor_tensor(sh, bpsum[:, 0:1], sc, ALU.mult)
                    nc.vector.tensor_tensor(sh, beta[:, ci:ci + 1], sh, ALU.subtract)
                    if ci in (0, 1):
                        nc.gpsimd.tensor_scalar(
                            out=h8[:, ci, :], in0=x_sb[:, ci, :],
                            scalar1=sc, scalar2=sh, op0=ALU.mult, op1=ALU.add,
                        )

                # chunk 2 stats split: sum(x) on Pool, sum(x^2) on ACT (one
                # Square pass, so the GN sqrt chain is barely delayed); DVE
                # keeps bn_stats for chunks 0/1/3.  Scratch writes land in
                # h8[:,2,:] / k8[:,2,:512*2], later overwritten.
                s12 = gpool.tile([P, 2], F32, tag="s12")
                nc.gpsimd.tensor_scalar(
                    out=h8[:, 2, :], in0=x_sb[:, 2, :],
                    scalar1=1.0, scalar2=None, op0=ALU.mult, op1=ALU.add,
                    accum_out=s12[:, 0:1],
                )
                nc.scalar.activation(
                    out=k8[:, 2, :HW], in_=x_sb[:, 2, :],
                    func=ACTF.Square, accum_out=s12[:, 1:2],
                )
                t2c2 = gpool.tile([P, 2], F32, tag="t2c2")
                nc.vector.tensor_scalar_mul(t2c2[:], s12[:], 1.0 / HW)

                for ci in (0, 1, 3):
                    stats = gpool.tile([P, HW // 512, 6], F32, tag="stats")
                    for sg in range(HW // 512):
                        nc.vector.bn_stats(
                            out=stats[:, sg, :],
                            in_=x_sb[:, ci, sg * 512:(sg + 1) * 512],
                        )
                    mv = gpool.tile([P, 2], F32, tag="mv")
                    nc.vector.bn_aggr(out=mv[:], in_=stats[:])
                    t2 = gpool.tile([P, 2], F32, tag="t2")
                    nc.vector.tensor_copy(out=t2[:, 0:1], in_=mv[:, 0:1])
                    nc.vector.tensor_tensor(
                        t2[:, 1:2], mv[:, 0:1], mv[:, 0:1], ALU.mult
                    )
                    nc.vector.tensor_add(t2[:, 1:2], t2[:, 1:2], mv[:, 1:2])
                    finish_chunk(ci, t2)
                    if ci == 0:
                        finish_chunk(2, t2c2)
                # h8 for chunks 2,3 emitted last: DVE picks them up right
                # after its bn_stats stream ends; half of c3 goes to Pool
                nc.vector.tensor_scalar(
                    out=h8[:, 2, :], in0=x_sb[:, 2, :],
                    scalar1=sc_all[:, 2:3], scalar2=sh_all[:, 2:3],
                    op0=ALU.mult, op1=ALU.add,
                )
                nc.vector.tensor_scalar(
                    out=h8[:, 3, :HW // 2], in0=x_sb[:, 3, :HW // 2],
                    scalar1=sc_all[:, 3:4], scalar2=sh_all[:, 3:4],
                    op0=ALU.mult, op1=ALU.add,
                )
                nc.gpsimd.tensor_scalar(
                    out=h8[:, 3, HW // 2:], in0=x_sb[:, 3, HW // 2:],
                    scalar1=sc_all[:, 3:4], scalar2=sh_all[:, 3:4],
                    op0=ALU.mult, op1=ALU.add,
                )
                # preload the exp activation table after the last Sqrt (the
                # input dep on sc_all pins it there despite list scheduling)
                expwarm = gpool.tile([P, 1], F32, tag="expwarm")
                nc.scalar.activation(
                    out=expwarm[:], in_=sc_all[:, 3:4], func=ACTF.Exp, scale=1.0,
                )

            # ====== convs + attention (fused pipeline, all fp8 DoubleRow) =====
            # Paired-exp layout: S for a j-chunk pair lands in one [P,2,512]
            # 2-bank psum tile, ONE [128x1024] exp per pair -> et ping-pong
            # buffer in SBUF.  AV runs as two bank-cycling passes over the
            # 2 O-banks: pass1 = cc0/cc1 (trailing the exps), pass2 = cc2/cc3
            # (burst during the next block's window, reading the et buffer).
            # PSUM: pp-pairs 2x2 + oA + oB + l + lrb = 8 banks.
            with (
                tc.tile_pool(name="etf", bufs=2) as etfpool,
                tc.tile_pool(name="o8b", bufs=2) as o8pool,
                tc.tile_pool(name="lb", bufs=2) as lbpool,
                tc.tile_pool(name="xpb", bufs=4) as xpbpool,
                tc.tile_pool(name="yt", bufs=4) as ytpool,
                tc.tile_pool(name="pp", bufs=2, space="PSUM") as pppool,
                tc.tile_pool(name="oap", bufs=1, space="PSUM") as oapool,
                tc.tile_pool(name="lp", bufs=1, space="PSUM") as lpool,
                tc.tile_pool(name="lrp", bufs=1, space="PSUM") as lrpool,
            ):
                etfs = [
                    etfpool.tile([P, NJC, 512], FP8, tag="etf", name=f"etf{b}")
                    for b in range(2)
                ]
                oA = oapool.tile([P, 512], F32, tag="oA")
                oB = oapool.tile([P, 512], F32, tag="oB")

                def emit_q(ib):
                    isl = slice(ib * 512, (ib + 1) * 512)
                    for pr in range(2):
                        ps = pppool.tile([P, 2, 512], F32, tag="pp",
                                         name=f"q{ib}{pr}")
                        for par in range(2):
                            oc = 2 * pr + par
                            for p2 in range(NP2):
                                nc.tensor.matmul(
                                    ps[:, par, :],
                                    lhsT=wq8[:, 2 * p2:2 * p2 + 2, oc * P:(oc + 1) * P],
                                    rhs=h8[:, 2 * p2:2 * p2 + 2, isl],
                                    start=(p2 == 0), stop=(p2 == NP2 - 1),
                                    perf_mode=DR,
                                )
                        for par in range(2):
                            oc = 2 * pr + par
                            eng = nc.gpsimd if par == 0 else nc.vector
                            eng.tensor_scalar(
                                out=q8[:, oc, isl], in0=ps[:, par, :],
                                scalar1=bq16[:, oc:oc + 1], scalar2=None,
                                op0=ALU.add,
                            )

                def emit_k(jt, pr):
                    tsl = slice(jt * 512, (jt + 1) * 512)
                    ps = pppool.tile([P, 2, 512], F32, tag="pp", name=f"k{jt}{pr}")
                    for par in range(2):
                        oc = 2 * pr + par
                        for p2 in range(NP2):
                            nc.tensor.matmul(
                                ps[:, par, :],
                                lhsT=wk8[:, 2 * p2:2 * p2 + 2, oc * P:(oc + 1) * P],
                                rhs=h8[:, 2 * p2:2 * p2 + 2, tsl],
                                start=(p2 == 0), stop=(p2 == NP2 - 1),
                                perf_mode=DR,
                            )
                    for par in range(2):
                        oc = 2 * pr + par
                        eng = nc.vector if par == 0 else nc.gpsimd
                        eng.tensor_scalar(
                            out=k8[:, oc, tsl], in0=ps[:, par, :],
                            scalar1=bk16[:, oc:oc + 1], scalar2=None, op0=ALU.add,
                        )

                def emit_v(vp):
                    ps = pppool.tile([P, 2, 512], F32, tag="pp", name=f"v{vp}")
                    for par in range(2):
                        jc = 2 * vp + par
                        for p2 in range(NP2):
                            nc.tensor.matmul(
                                ps[:, par, :],
                                lhsT=h8[:, 2 * p2:2 * p2 + 2, jc * P:(jc + 1) * P],
                                rhs=wv8[:, 2 * p2:2 * p2 + 2, :],
                                start=(p2 == 0), stop=(p2 == NP2 - 1),
                                perf_mode=DR,
                            )
                    eng = nc.gpsimd if vp % 2 == 0 else nc.vector
                    eng.tensor_copy(out=vt8[:, 2 * vp:2 * vp + 2, :], in_=ps[:])

                def emit_proj(ib, o8t):
                    isl = slice(ib * 512, (ib + 1) * 512)
                    for pr in range(2):
                        ps = pppool.tile([P, 2, 512], F32, tag="pp",
                                         name=f"p{ib}{pr}")
                        for par in range(2):
                            oc = 2 * pr + par
                            for p2 in range(NP2):
                                nc.tensor.matmul(
                                    ps[:, par, :],
                                    lhsT=wp8[:, 2 * p2:2 * p2 + 2, oc * P:(oc + 1) * P],
                                    rhs=o8t[:, 2 * p2:2 * p2 + 2, :],
                                    start=(p2 == 0), stop=(p2 == NP2 - 1),
                                    perf_mode=DR,
                                )
                        for par in range(2):
                            oc = 2 * pr + par
                            xpb = xpbpool.tile([P, 512], F32, tag="xpb",
                                               name=f"xpb{ib}{oc}")
                            nc.gpsimd.tensor_scalar(
                                out=xpb[:], in0=x_sb[:, oc, isl],
                                scalar1=bppc[:, oc:oc + 1], scalar2=None,
                                op0=ALU.add,
                            )
                            eng = nc.vector if par == 0 else nc.gpsimd
                            yt = ytpool.tile([P, 512], F32, tag="yt",
                                             name=f"yt{ib}{oc}")
                            eng.scalar_tensor_tensor(
                                out=yt[:], in0=ps[:, par, :], scalar=PROJ_SCALE,
                                in1=xpb[:], op0=ALU.mult, op1=ALU.add,
                            )
                            if ib == NBLK - 1:
                                deng = (nc.sync, nc.gpsimd, nc.scalar, nc.sync)[oc]
                            else:
                                deng = nc.sync
                            deng.dma_start(out=y_d[oc * P:(oc + 1) * P, isl],
                                           in_=yt[:])

                def emit_s(ib, jp):
                    isl = slice(ib * 512, (ib + 1) * 512)
                    etf = etfs[ib % 2]
                    ps = pppool.tile([P, 2, 512], F32, tag="pp",
                                     name=f"s{ib}_{jp}")
                    for par in range(2):
                        jc = 2 * jp + par
                        for p2 in range(NP2):
                            nc.tensor.matmul(
                                ps[:, par, :],
                                lhsT=k8[:, 2 * p2:2 * p2 + 2, jc * P:(jc + 1) * P],
                                rhs=q8[:, 2 * p2:2 * p2 + 2, isl],
                                start=(p2 == 0), stop=(p2 == NP2 - 1),
                                perf_mode=DR,
                            )
                    nc.scalar.activation(
                        out=etf[:, 2 * jp:2 * jp + 2, :], in_=ps[:],
                        func=ACTF.Exp, scale=SCALE_S, bias=ebias[:],
                    )

                def emit_av(ib, jp, ccs, dests):
                    etf = etfs[ib % 2]
                    for cc, dest in zip(ccs, dests):
                        nc.tensor.matmul(
                            dest[:],
                            lhsT=vt8[:, 2 * jp:2 * jp + 2, cc * P:(cc + 1) * P],
                            rhs=etf[:, 2 * jp:2 * jp + 2, :],
                            start=(jp == 0), stop=(jp == NJP - 1),
                            perf_mode=DR,
                        )

                def emit_l(ib, jp, lt):
                    etf = etfs[ib % 2]
                    nc.tensor.matmul(
                        lt[0:1, :], lhsT=ones8[:],
                        rhs=etf[:, 2 * jp:2 * jp + 2, :],
                        start=(jp == 0), stop=(jp == NJP - 1),
                        perf_mode=DR,
                    )

                lts = {}
                lrbs = {}
                o8ts = {}
                l_next = [0] * NBLK
                av1_next = [0] * NBLK
                cc2_next = 0

                def lrb_chain(ib):
                    l_bf = lbpool.tile([1, 512], BF16, tag="lbf", name=f"lbf{ib}")
                    with nc.allow_low_precision(reason="1/l bcast bf16; 0.4% on a 6.5%-of-norm term"):
                        nc.vector.reciprocal(out=l_bf[:], in_=lts[ib][0:1, :])
                    if ib < NBLK - 1:
                        lrbps = lrpool.tile([P, 512], F32, tag="lr",
                                            name=f"lrb{ib}")
                    else:
                        # last block: broadcast into a pp-ring pair half
                        lrbps = pppool.tile([P, 2, 512], F32, tag="pp",
                                            name="lrb3t")[:, 0, :]
                    nc.tensor.matmul(
                        lrbps[:], lhsT=ones_bf[:], rhs=l_bf[:],
                        start=True, stop=True,
                    )
                    lrbs[ib] = lrbps

                def o8_front(ib):
                    o8t = o8pool.tile([P, NCC, 512], FP8, tag="o8",
                                      name=f"o8_{ib}")
                    o8ts[ib] = o8t
                    nc.vector.tensor_tensor(
                        o8t[:, 0, :], oA[:], lrbs[ib][:], ALU.mult)
                    nc.gpsimd.tensor_tensor(
                        o8t[:, 1, :], oB[:], lrbs[ib][:], ALU.mult)

                def o8_back(ib):
                    o8t = o8ts[ib]
                    nc.vector.tensor_tensor(
                        o8t[:, 2, :], oA[:], lrbs[ib][:], ALU.mult)
                    nc.gpsimd.tensor_tensor(
                        o8t[:, 3, :], oB[:], lrbs[ib][:], ALU.mult)

                def pump_l(w, upto):
                    while l_next[w] <= min(upto, NJP - 1):
                        emit_l(w, l_next[w], lts[w])
                        l_next[w] += 1

                def pump_av1(w, upto, batch=99):
                    n = 0
                    while av1_next[w] <= min(upto, NJP - 1) and n < batch:
                        emit_av(w, av1_next[w], (0, 1), (oA, oB))
                        av1_next[w] += 1
                        n += 1

                # ---------------- prologue ----------------
                emit_k(0, 0)
                emit_k(0, 1)
                emit_q(0)
                emit_v(0)
                emit_q(1)

                lts[0] = lpool.tile([P, 512], F32, tag="l", name="l0")
                # ---------------- block 0 (convs JIT, AV1 trails by 2) -------
                for jp in range(NJP):
                    jt, half = divmod(jp + 2, 2)
                    if jt <= NJT - 1:
                        emit_k(jt, half)
                    if jp + 1 < NJP:
                        emit_v(jp + 1)
                    emit_s(0, jp)
                    pump_av1(0, jp - 2)
                    pump_l(0, jp - 2)
                    if jp == 8:
                        emit_q(2)
                # ---------------- blocks 1..3 ----------------
                cc2ps = None
                for w in range(1, NBLK):
                    ib = w - 1          # predecessor draining this window
                    last = w == NBLK - 1
                    for t in range(NJP):
                        emit_s(w, t)
                        if t == 0:
                            pump_av1(ib, NJP - 1)   # drain leftovers
                            pump_l(ib, NJP - 1)
                        elif t == 1:
                            lrb_chain(ib)
                            lts[w] = lpool.tile([P, 512], F32, tag="l",
                                                name=f"l{w}")
                        elif t == 2:
                            o8_front(ib)
                        elif 3 <= t <= 8:
                            # pass2 of predecessor: ~3 j-pairs per slot
                            for j in range(3 * (t - 3), min(3 * (t - 2), NJP)):
                                emit_av(ib, j, (2, 3), (oA, oB))
                        elif t == 9:
                            o8_back(ib)
                        elif t == 10:
                            emit_proj(ib, o8ts[ib])
                        elif t >= 11:
                            pump_av1(w, t - 1, batch=4)
                            if last:
                                if cc2ps is None:
                                    cc2ps = lrpool.tile([P, 512], F32, tag="lr",
                                                        name="cc2ib3")
                                n = 0
                                while cc2_next <= t - 1 and n < 3:
                                    emit_av(w, cc2_next, (2,), (cc2ps,))
                                    cc2_next += 1
                                    n += 1
                        if t >= 2:
                            pump_l(w, t - 2)
                        if t == 8 and w == 1:
                            emit_q(3)

                # ---------------- tail (after last exp) ----------------
                w = NBLK - 1
                pump_av1(w, NJP - 1)
                pump_l(w, NJP - 1)
                while cc2_next < NJP:
                    emit_av(w, cc2_next, (2,), (cc2ps,))
                    cc2_next += 1
                lrb_chain(w)
                o8_front(w)
                o8t = o8ts[w]
                nc.gpsimd.tensor_tensor(
                    o8t[:, 2, :], cc2ps[:], lrbs[w][:], ALU.mult)
                # cc3 burst into the freed oA bank
                for j in range(NJP):
                    nc.tensor.matmul(
                        oA[:],
                        lhsT=vt8[:, 2 * j:2 * j + 2, 3 * P:4 * P],
                        rhs=etfs[w % 2][:, 2 * j:2 * j + 2, :],
                        start=(j == 0), stop=(j == NJP - 1),
                        perf_mode=DR,
                    )
                nc.vector.tensor_tensor(
                    o8t[:, 3, :], oA[:], lrbs[w][:], ALU.mult)
                emit_proj(w, o8t)

    if split_waits:
        _split_excess_waits(nc)
    return nc


_NC = None


def _get_nc():
    global _NC
    if _NC is None:
        _NC = build_nc()
    return _NC


def _core0_feed(inputs):
    """Input map for core 0 (batch 0, first query half) — used by test harnesses."""
    maps = _build_in_maps(**inputs)
    return maps[0]


def _build_in_maps(x, gamma, beta, Wq, bq, Wk, bk, Wv, bv, Wp, bp):
    x = np.asarray(x, dtype=np.float32)
    B, c, H, W = x.shape
    assert (B, c, H, W) == (4, C, 64, 64)

    def pc(v):  # [C] -> [P, NCC]
        return np.ascontiguousarray(np.asarray(v, np.float32).reshape(NCC, P).T)

    ind16 = np.zeros((P, P // GS), np.float32)
    ind16[np.arange(P), np.arange(P) // GS] = 1.0 / GS
    bcast16 = np.zeros((P // GS, P), np.float32)
    bcast16[np.arange(P) // GS, np.arange(P)] = 1.0

    bp_eff = np.asarray(bp, np.float64) + np.asarray(Wp, np.float64) @ np.asarray(bv, np.float64)
    consts = np.concatenate(
        [pc(16.0 * np.asarray(bq, np.float32)),
         pc(16.0 * np.asarray(bk, np.float32)),
         pc(bp_eff.astype(np.float32)),
         pc(gamma), pc(beta), ind16], axis=1,
    ).astype(np.float32)

    def w8(w):
        return np.ascontiguousarray(16.0 * np.asarray(w, np.float32).T).astype(E4)

    shared = {
        "wq8": w8(Wq), "wk8": w8(Wk), "wv8": w8(Wv), "wp8": w8(Wp),
        "consts": np.ascontiguousarray(consts),
        "bcast16": bcast16,
        "ones8": np.full((P, 2, 1), ONES_VAL, E4),
    }

    xf = x.reshape(B, C, HW)
    in_maps = []
    for core in range(8):
        b, half = divmod(core, 2)
        xb = xf[b]
        if half == 0:
            x_bc = xb
        else:
            x_bc = np.concatenate([xb[:, IHALF:], xb[:, :IHALF]], axis=1)
        in_maps.append({"x_bf": np.ascontiguousarray(x_bc).astype(BF), **shared})
    return in_maps


def kernel(x, gamma, beta, Wq, bq, Wk, bk, Wv, bv, Wp, bp):
    nc = _get_nc()
    in_maps = _build_in_maps(x, gamma, beta, Wq, bq, Wk, bk, Wv, bv, Wp, bp)

    from concourse.bass_utils import run_bass_kernel_spmd

    res = run_bass_kernel_spmd(nc, in_maps, list(range(8)))

    B = 4
    out = np.empty((B, C, HW), np.float32)
    for core in range(8):
        b, half = divmod(core, 2)
        out[b, :, half * IHALF:(half + 1) * IHALF] = res.results[core]["yout"]
    return out.reshape(B, C, 64, 64)
